# revision 10
# baseline (speedup 1.0000x reference)
"""AttentionBlock (GroupNorm -> 1x1 qkv conv -> spatial attention with
softmax over the last width axis -> 1x1 proj conv -> residual) on 8
Trainium2 NeuronCores, data-parallel over the batch.

Self-contained: hardcodes shapes B,C,H,W = 32,512,32,32 and the
8-core batch sharding. Host-side preprocessing folds the C**-0.25
attention scale into the q/k weight rows and the v-projection bias into
the proj bias (sum_ij softmax_j(S) == H exactly), so the device kernel
is: GN stats (bn_stats over a (sample,group)-partitioned view), affine
fold, and a chain of PE matmuls with the softmax done in "scores
transposed" orientation (ij on partitions). Cross-partition softmax
sums/broadcasts are PE matmuls with 0/1 select matrices; 1/x and
rsqrt run on ACT via exp/ln (single activation-table set).
"""

import os
from contextlib import ExitStack

import numpy as np

B, C, H, W = 32, 512, 32, 32
HW = H * W            # 1024
G = 32                # groupnorm groups
GS = C // G           # 16 channels per group
NCORES = 8
BS = B // NCORES      # 4 samples per core
EPS = 1e-5
P = 128
CT = C // P           # 4 channel tiles
IJT = HW // P         # 8 key-pixel tiles
NF = 512              # matmul moving free dim
NH = HW // NF         # 2

# "f32"  : accurate fp32 matmuls (4 cycles/row on PE)
# "f32r" : single-pass fp32 matmuls (1 cycle/row, reduced precision)
MM_MODE = os.environ.get("ATTN_MM_MODE", "f32")

_cache: dict = {}


def _build(mm_mode: str):
    import concourse.bass as bass
    import concourse.tile as tile
    from concourse import bacc, mybir

    dt = mybir.dt
    AF = mybir.ActivationFunctionType
    ALU = mybir.AluOpType
    f32 = dt.float32
    mmdt = {"f32": f32, "f32r": dt.float32r}[mm_mode]

    def mm(ap):
        return ap.bitcast(mmdt) if mmdt != f32 else ap

    nc = bacc.Bacc("TRN2", target_bir_lowering=False, debug=False)

    x_d = nc.dram_tensor("x", [BS, C, HW], f32, kind="ExternalInput").ap()
    wt_d = nc.dram_tensor("wt", [C, 3 * C], f32, kind="ExternalInput").ap()
    pt_d = nc.dram_tensor("pt", [C, C], f32, kind="ExternalInput").ap()
    qkb_d = nc.dram_tensor("qkb", [P, 8], f32, kind="ExternalInput").ap()
    pb_d = nc.dram_tensor("pb", [P, CT], f32, kind="ExternalInput").ap()
    gw_d = nc.dram_tensor("gw", [P, GS], f32, kind="ExternalInput").ap()
    gb_d = nc.dram_tensor("gb", [P, GS], f32, kind="ExternalInput").ap()
    ss_d = nc.dram_tensor("ssum", [P, IJT * G], f32, kind="ExternalInput").ap()
    sb_d = nc.dram_tensor("sbc", [G, IJT * P], f32, kind="ExternalInput").ap()
    out_d = nc.dram_tensor("out", [BS, C, HW], f32, kind="ExternalOutput").ap()

    with tile.TileContext(nc) as tc, ExitStack() as ctx:
        singles = ctx.enter_context(tc.tile_pool(name="singles", bufs=1))
        gnp = ctx.enter_context(tc.tile_pool(name="gnp", bufs=2))
        small = ctx.enter_context(tc.tile_pool(name="small", bufs=2))
        ptmp = ctx.enter_context(tc.tile_pool(name="ptmp", bufs=4))
        bigs = ctx.enter_context(tc.tile_pool(name="bigs", bufs=1))
        pmm = ctx.enter_context(tc.tile_pool(name="pmm", bufs=4, space="PSUM"))
        pdp = ctx.enter_context(tc.tile_pool(name="pdp", bufs=2, space="PSUM"))
        prb = ctx.enter_context(tc.tile_pool(name="prb", bufs=2, space="PSUM"))

        # ---- constants ----
        wt_sb = singles.tile([P, CT, 3 * C], f32)
        nc.sync.dma_start(wt_sb, wt_d.rearrange("(k p) o -> p k o", p=P))
        pt_sb = singles.tile([P, CT, C], f32)
        nc.sync.dma_start(pt_sb, pt_d.rearrange("(k p) o -> p k o", p=P))
        qkb_sb = singles.tile([P, 8], f32)
        nc.sync.dma_start(qkb_sb, qkb_d)
        pb_sb = singles.tile([P, CT], f32)
        nc.sync.dma_start(pb_sb, pb_d)
        gw_sb = singles.tile([P, GS], f32)
        nc.sync.dma_start(gw_sb, gw_d)
        gb_sb = singles.tile([P, GS], f32)
        nc.sync.dma_start(gb_sb, gb_d)
        ss_sb = singles.tile([P, IJT, G], f32)
        nc.sync.dma_start(ss_sb, ss_d.rearrange("p (t g) -> p t g", t=IJT))
        sbc_sb = singles.tile([G, IJT * P], f32)
        nc.sync.dma_start(sbc_sb, sb_d)

        # ---- GroupNorm stats for all BS samples at once ----
        # partition p = (sample s)*32 + (group g); free = the group's
        # 16 channels x 1024 pixels = 16384 contiguous elements.
        xf = x_d.rearrange("b (g u) f -> (b g) (u f)", g=G)
        stats = singles.tile([P, G, 6], f32)
        for ch in range(4):
            xg = gnp.tile([P, 4096], f32, tag="xg")
            nc.gpsimd.dma_start(xg, xf[:, ch * 4096 : (ch + 1) * 4096])
            for sub in range(8):
                nc.vector.bn_stats(
                    out=stats[:, ch * 8 + sub, :],
                    in_=xg[:, sub * 512 : (sub + 1) * 512],
                )
        mv = singles.tile([P, 2], f32)
        nc.vector.bn_aggr(out=mv, in_=stats)
        # rstd = exp(-0.5*ln(var+eps)); ln/exp share one ACT table set
        epsb = singles.tile([P, 1], f32)
        nc.vector.memset(epsb, EPS)
        rstd = singles.tile([P, 1], f32)
        nc.scalar.activation(out=rstd, in_=mv[:, 1:2], func=AF.Ln, bias=epsb[:, 0:1])
        nc.scalar.activation(out=rstd, in_=rstd, func=AF.Exp, scale=-0.5)
        # per-channel scale/offset in (s,g) layout:
        # sA = rstd*w ; tA = b - mean*sA
        sA = singles.tile([P, GS], f32)
        nc.vector.tensor_scalar_mul(sA, gw_sb, rstd[:, 0:1])
        tA = singles.tile([P, GS], f32)
        nc.vector.tensor_scalar(tA, sA, mv[:, 0:1], None, op0=ALU.mult)
        nc.vector.tensor_tensor(tA, gb_sb, tA, ALU.subtract)
        # scatter to channel-partitioned per-sample scale vectors
        scv = singles.tile([P, BS * CT], f32)
        tcv = singles.tile([P, BS * CT], f32)
        for s in range(BS):
            for q in range(CT):
                col = s * CT + q
                base = s * G + q * 8
                nc.sync.dma_start(scv[:, col : col + 1], sA[base : base + 8, :])
                nc.sync.dma_start(tcv[:, col : col + 1], tA[base : base + 8, :])

        # ---- per-sample attention ----
        for s in range(BS):
            xv = x_d[s].rearrange("(q p) f -> p q f", p=P)
            ov = out_d[s].rearrange("(q p) f -> p q f", p=P)

            xt = bigs.tile([P, CT, HW], f32, tag="xt")
            nc.gpsimd.dma_start(xt, xv)
            nt = bigs.tile([P, CT, HW], f32, tag="nt")
            for q in range(CT):
                nc.vector.tensor_scalar(
                    nt[:, q],
                    xt[:, q],
                    scv[:, s * CT + q : s * CT + q + 1],
                    tcv[:, s * CT + q : s * CT + q + 1],
                    op0=ALU.mult,
                    op1=ALU.add,
                )

            # q' and k' [c, hw] (scale folded into weights, bias via ACT)
            qksb = bigs.tile([P, 8, HW], f32, tag="qk")
            for ot in range(8):
                for n in range(NH):
                    ps = pmm.tile([P, NF], f32, tag="mm")
                    for k in range(CT):
                        nc.tensor.matmul(
                            ps,
                            lhsT=mm(wt_sb[:, k, ot * P : (ot + 1) * P]),
                            rhs=mm(nt[:, k, n * NF : (n + 1) * NF]),
                            start=(k == 0),
                            stop=(k == CT - 1),
                        )
                    nc.scalar.activation(
                        out=qksb[:, ot, n * NF : (n + 1) * NF],
                        in_=ps,
                        func=AF.Identity,
                        bias=qkb_sb[:, ot : ot + 1],
                    )

            # v computed transposed: [ij, c_v] (v bias folded into proj bias)
            vtsb = bigs.tile([P, IJT, C], f32, tag="vt")
            for t in range(IJT):
                ps = pmm.tile([P, NF], f32, tag="mm")
                for k in range(CT):
                    nc.tensor.matmul(
                        ps,
                        lhsT=mm(nt[:, k, t * P : (t + 1) * P]),
                        rhs=mm(wt_sb[:, k, 2 * C : 3 * C]),
                        start=(k == 0),
                        stop=(k == CT - 1),
                    )
                nc.vector.tensor_copy(out=vtsb[:, t], in_=ps)

            # scores transposed S^T[ij, hw]; E = exp(S^T); per-(i,hw)
            # denominators D via select-matrix matmuls (sum 32 j-partitions,
            # accumulating all 8 ij-tiles into one [32, NF] psum)
            esb = bigs.tile([P, IJT, HW], f32, tag="E")
            dd = small.tile([G, HW], f32, tag="dd")
            for t in range(IJT):
                for n in range(NH):
                    ps = pmm.tile([P, NF], f32, tag="mm")
                    for k in range(CT):
                        nc.tensor.matmul(
                            ps,
                            lhsT=mm(qksb[:, 4 + k, t * P : (t + 1) * P]),
                            rhs=mm(qksb[:, k, n * NF : (n + 1) * NF]),
                            start=(k == 0),
                            stop=(k == CT - 1),
                        )
                    nc.scalar.activation(
                        out=esb[:, t, n * NF : (n + 1) * NF], in_=ps, func=AF.Exp
                    )
            for n in range(NH):
                pd = pdp.tile([G, NF], f32, tag="pd")
                for t in range(IJT):
                    nc.tensor.matmul(
                        pd,
                        lhsT=mm(ss_sb[:, t, :]),
                        rhs=mm(esb[:, t, n * NF : (n + 1) * NF]),
                        start=(t == 0),
                        stop=(t == IJT - 1),
                    )
                nc.vector.tensor_copy(out=dd[:, n * NF : (n + 1) * NF], in_=pd)

            # R = 1/D via exp(-ln(D)) on ACT
            rr = small.tile([G, HW], f32, tag="rr")
            nc.scalar.activation(out=rr, in_=dd, func=AF.Ln)
            nc.scalar.activation(out=rr, in_=rr, func=AF.Exp, scale=-1.0)

            # A^T = E * broadcast(R): PE broadcast (i -> 32 j-partitions)
            for t in range(IJT):
                for n in range(NH):
                    pb_ps = prb.tile([P, NF], f32, tag="rb")
                    nc.tensor.matmul(
                        pb_ps,
                        lhsT=mm(sbc_sb[:, t * P : (t + 1) * P]),
                        rhs=mm(rr[:, n * NF : (n + 1) * NF]),
                        start=True,
                        stop=True,
                    )
                    nc.vector.tensor_tensor(
                        esb[:, t, n * NF : (n + 1) * NF],
                        esb[:, t, n * NF : (n + 1) * NF],
                        pb_ps,
                        ALU.mult,
                    )

            # h[c, hw] = sum_ij v^T[ij,c] * A^T[ij,hw]  (h overwrites nt)
            for ct in range(CT):
                for n in range(NH):
                    ps = pmm.tile([P, NF], f32, tag="mm")
                    for t in range(IJT):
                        nc.tensor.matmul(
                            ps,
                            lhsT=mm(vtsb[:, t, ct * P : (ct + 1) * P]),
                            rhs=mm(esb[:, t, n * NF : (n + 1) * NF]),
                            start=(t == 0),
                            stop=(t == IJT - 1),
                        )
                    nc.scalar.activation(
                        out=nt[:, ct, n * NF : (n + 1) * NF], in_=ps, func=AF.Identity
                    )

            # proj + bias + residual (accumulated into xt), then store
            for ot in range(CT):
                for n in range(NH):
                    ps = pmm.tile([P, NF], f32, tag="mm")
                    for k in range(CT):
                        nc.tensor.matmul(
                            ps,
                            lhsT=mm(pt_sb[:, k, ot * P : (ot + 1) * P]),
                            rhs=mm(nt[:, k, n * NF : (n + 1) * NF]),
                            start=(k == 0),
                            stop=(k == CT - 1),
                        )
                    tmp = ptmp.tile([P, NF], f32, tag="pt")
                    nc.scalar.activation(
                        out=tmp, in_=ps, func=AF.Identity, bias=pb_sb[:, ot : ot + 1]
                    )
                    nc.vector.tensor_tensor(
                        xt[:, ot, n * NF : (n + 1) * NF],
                        xt[:, ot, n * NF : (n + 1) * NF],
                        tmp,
                        ALU.add,
                    )
            nc.gpsimd.dma_start(ov, xt)

    nc.compile()
    return nc


def _prep_inputs(x, gn_w, gn_b, qkv_w, qkv_b, proj_w, proj_b):
    x = np.asarray(x, dtype=np.float32)
    gn_w = np.asarray(gn_w, dtype=np.float32)
    gn_b = np.asarray(gn_b, dtype=np.float32)
    qkv_w = np.asarray(qkv_w, dtype=np.float32)
    qkv_b = np.asarray(qkv_b, dtype=np.float32)
    proj_w = np.asarray(proj_w, dtype=np.float32)
    proj_b = np.asarray(proj_b, dtype=np.float32)

    s4 = np.float32(float(C) ** -0.25)
    w_s = qkv_w.copy()
    w_s[: 2 * C] *= s4
    wt = np.ascontiguousarray(w_s.T)                     # [C, 3C]
    pt = np.ascontiguousarray(proj_w.T)                  # [C, C]
    qkb = np.ascontiguousarray((qkv_b[: 2 * C] * s4).reshape(8, P).T)  # [P, 8]
    vb = qkv_b[2 * C :]
    pb = np.ascontiguousarray(
        (proj_b + np.float32(H) * (proj_w @ vb)).reshape(CT, P).T
    )                                                    # [P, CT]
    gw = np.ascontiguousarray(np.tile(gn_w.reshape(G, GS), (BS, 1)))  # [P, GS]
    gb = np.ascontiguousarray(np.tile(gn_b.reshape(G, GS), (BS, 1)))
    ss = np.zeros((P, IJT, G), dtype=np.float32)
    for t in range(IJT):
        for p in range(P):
            ss[p, t, 4 * t + p // 32] = 1.0
    ss = np.ascontiguousarray(ss.reshape(P, IJT * G))
    sbc = np.zeros((G, IJT, P), dtype=np.float32)
    for t in range(IJT):
        for p in range(P):
            sbc[4 * t + p // 32, t, p] = 1.0
    sbc = np.ascontiguousarray(sbc.reshape(G, IJT * P))

    shared = {
        "wt": wt, "pt": pt, "qkb": qkb, "pb": pb,
        "gw": gw, "gb": gb, "ssum": ss, "sbc": sbc,
    }
    in_maps = []
    for c in range(NCORES):
        m = dict(shared)
        m["x"] = np.ascontiguousarray(x[c * BS : (c + 1) * BS].reshape(BS, C, HW))
        in_maps.append(m)
    return in_maps


def run(inputs: dict, trace: bool = False, n_cores: int = NCORES):
    """Build (cached), run on hardware, return (results, BassKernelResults)."""
    from concourse.bass_utils import run_bass_kernel_spmd

    key = MM_MODE
    if key not in _cache:
        _cache[key] = _build(MM_MODE)
    nc = _cache[key]
    in_maps = _prep_inputs(**inputs)[:n_cores]
    res = run_bass_kernel_spmd(nc, in_maps, list(range(n_cores)), trace=trace)
    return res


def kernel(x, gn_w, gn_b, qkv_w, qkv_b, proj_w, proj_b) -> np.ndarray:
    res = run(dict(x=x, gn_w=gn_w, gn_b=gn_b, qkv_w=qkv_w, qkv_b=qkv_b,
                   proj_w=proj_w, proj_b=proj_b))
    out = np.concatenate(
        [res.results[c]["out"].reshape(BS, C, H, W) for c in range(NCORES)], axis=0
    )
    return out


# revision 39
# speedup vs baseline: 78.3107x; 78.3107x over previous
"""AttentionBlock (GroupNorm -> 1x1 qkv conv -> spatial attention with
softmax over the last width axis -> 1x1 proj conv -> residual) on 8
Trainium2 NeuronCores, data-parallel over the batch.

Self-contained: hardcodes shapes B,C,H,W = 32,512,32,32 and the
8-core batch sharding. Host-side preprocessing folds the C**-0.25
attention scale into the q/k weight rows, transposes the 1x1-conv
weights, and folds the v bias into the proj bias (sum_ij softmax_j(S)
== H exactly). On-device, per sample: GroupNorm stats via channel-wise
bn_stats + a tiny PE select-matmul for the 16-channel group combine
(rsqrt = DVE quake-seed Newton, so only the Exp ACT table is ever
loaded); qkv/scores/attn@v/proj as PE matmuls with the softmax done in
"scores transposed" orientation (ij on partitions, v computed
transposed directly): softmax-over-j denominators are a select-matrix
PE matmul, 1/D is a custom DVE approx op, and the i->32-j-partition
broadcast is a replicating DMA with the A-multiply on GPSIMD. The
emission is software-pipelined (next sample's qkv/vT matmuls are
emitted inside this sample's softmax window) so the PE stays ~97%
busy. MM_MODE env ATTN_MM_MODE: f32 (default, rel err ~2e-6),
mix (~1.4e-4, 1.43x faster), f32r (~2.7e-4, 2.96x faster).
"""

import os
from contextlib import ExitStack

import numpy as np

B, C, H, W = 32, 512, 32, 32
HW = H * W            # 1024
G = 32                # groupnorm groups
GS = C // G           # 16 channels per group
NCORES = 8
BS = B // NCORES      # 4 samples per core
EPS = 1e-5
P = 128
CT = C // P           # 4 channel tiles
IJT = HW // P         # 8 key-pixel tiles
NF = 512              # matmul moving free dim
NH = HW // NF         # 2

# "f32"  : accurate fp32 matmuls (4 cycles/row on PE)
# "f32r" : single-pass fp32 matmuls (1 cycle/row, reduced precision)
MM_MODE = os.environ.get("ATTN_MM_MODE", "f32")

_cache: dict = {}


def _build(mm_mode: str):
    import concourse.bass as bass
    import concourse.tile as tile
    from concourse import bacc, mybir

    dt = mybir.dt
    AF = mybir.ActivationFunctionType
    ALU = mybir.AluOpType
    f32 = dt.float32
    f32r = dt.float32r
    # matmul-operand dtypes: float32r tiles make the producing engines round
    # to the PE's fast-fp32 format (verifier requires producer dtype match).
    # "mix" keeps the input/output projections fp32 and runs only the
    # attention-interior matmuls (scores, softmax-sum, attn@v) in f32r.
    if mm_mode == "f32":
        mdt = mdt_att = f32
    elif mm_mode == "f32r":
        mdt = mdt_att = f32r
    else:  # mix
        mdt, mdt_att = f32, f32r

    def mm(ap):
        return ap

    nc = bacc.Bacc("TRN2", target_bir_lowering=False, debug=False,
                   dynamic_dma_scratch_size=8192)

    x_d = nc.dram_tensor("x", [BS, C, HW], f32, kind="ExternalInput").ap()
    wt_d = nc.dram_tensor("wt", [C, 3 * C], mdt, kind="ExternalInput").ap()
    pt_d = nc.dram_tensor("pt", [C, C], mdt, kind="ExternalInput").ap()
    qkb_d = nc.dram_tensor("qkb", [P, 8], f32, kind="ExternalInput").ap()
    pb_d = nc.dram_tensor("pb", [P, CT], f32, kind="ExternalInput").ap()
    gw_d = nc.dram_tensor("gw", [P, CT], f32, kind="ExternalInput").ap()
    gb_d = nc.dram_tensor("gb", [P, CT], f32, kind="ExternalInput").ap()
    sg_d = nc.dram_tensor("selg", [P, 8], mdt, kind="ExternalInput").ap()
    ss_d = nc.dram_tensor("ssum", [P, IJT * G], mdt_att, kind="ExternalInput").ap()
    out_d = nc.dram_tensor("out", [BS, C, HW], f32, kind="ExternalOutput").ap()

    with tile.TileContext(nc) as tc, ExitStack() as ctx:
        singles = ctx.enter_context(tc.tile_pool(name="singles", bufs=1))
        pmm = ctx.enter_context(tc.tile_pool(name="pmm", bufs=6, space="PSUM"))
        pdp = ctx.enter_context(tc.tile_pool(name="pdp", bufs=2, space="PSUM"))

        def pbc(base, rep):
            # partition-broadcast source AP: replicate each source partition
            # `rep` times (destination iterates partitions major)
            base = base.opt(keep_dims={0})
            ap = [d for d in base.ap[1:] if d[1] > 1] or [[1, 1]]
            return bass.AP(
                tensor=base.tensor, offset=base.offset,
                ap=[base.ap[0], [0, rep], *ap],
            )

        # startup DMA priority: xt(0) first (gates GN stats + first matmul),
        # then small constants + the qkv weight, then the remaining x tiles
        # and the proj weight (needed much later)
        xtp = ctx.enter_context(tc.tile_pool(name="xtp", bufs=3))
        xts = {}
        xts[0] = xtp.tile([P, CT, HW], f32, tag="xt", name="xt0")
        for q in range(CT):
            nc.sync.dma_start(
                xts[0][:, q], x_d[0].rearrange("(q p) f -> p q f", p=P)[:, q]
            )

        qkb_sb = singles.tile([P, 8], f32)
        nc.scalar.dma_start(qkb_sb, qkb_d)
        pb_sb = singles.tile([P, CT], f32)
        nc.gpsimd.dma_start(pb_sb, pb_d)
        gw_sb = singles.tile([P, CT], f32)
        nc.gpsimd.dma_start(gw_sb, gw_d)
        gb_sb = singles.tile([P, CT], f32)
        nc.gpsimd.dma_start(gb_sb, gb_d)
        selg_sb = singles.tile([P, 8], mdt)
        nc.scalar.dma_start(selg_sb, sg_d)
        ss_sb = singles.tile([P, IJT, G], mdt_att)
        nc.gpsimd.dma_start(ss_sb, ss_d.rearrange("p (t g) -> p t g", t=IJT))

        wt_sb = singles.tile([P, CT, 3 * C], mdt)
        wtv = wt_d.rearrange("(k p) o -> p k o", p=P)
        for ot in range(12):
            nc.sync.dma_start(
                wt_sb[:, :, ot * P : (ot + 1) * P], wtv[:, :, ot * P : (ot + 1) * P]
            )

        small = ctx.enter_context(tc.tile_pool(name="small", bufs=1))
        stp = ctx.enter_context(tc.tile_pool(name="stp", bufs=2))
        epsb = singles.tile([P, 1], f32)
        nc.vector.memset(epsb, EPS)
        # warm the Exp ACT table set while the first DMAs run (the only
        # table-based ACT function this kernel uses)
        actwarm = singles.tile([P, 1], f32)
        nc.scalar.activation(out=actwarm, in_=epsb, func=AF.Exp)
        magic = singles.tile([8, CT, 1], dt.int32)
        nc.vector.memset(magic, 0x5F3759DF)
        # per-(sample,group) stats: [8 group-in-qtile, (s,q), (mean, E[x^2])]
        gst = singles.tile([8, BS * CT, 2], f32)
        scv = singles.tile([P, BS * CT], f32)
        tcv = singles.tile([P, BS * CT], f32)

        def emit_stats(s):
            """Channel bn_stats on xt(s) -> group combine on PE -> per-channel
            GN scale/offset columns scv/tcv[:, s*CT..]."""
            xt = xts[s]
            for q in range(CT):
                stq = stp.tile([P, 2, 6], f32, tag="stq")
                for sub in range(2):
                    nc.vector.bn_stats(
                        out=stq[:, sub, :], in_=xt[:, q, sub * 512 : (sub + 1) * 512]
                    )
                mvq = stp.tile([P, 2], f32, tag="mvq")
                nc.vector.bn_aggr(out=mvq, in_=stq)
                exq = stp.tile([P, 2], mdt, tag="exq")
                nc.vector.tensor_copy(out=exq[:, 0:1], in_=mvq[:, 0:1])
                nc.vector.tensor_scalar(
                    exq[:, 1:2], mvq[:, 0:1], mvq[:, 0:1], mvq[:, 1:2],
                    op0=ALU.mult, op1=ALU.add,
                )
                pg = pdp.tile([8, 2], f32, tag="pd")
                nc.tensor.matmul(pg, lhsT=selg_sb, rhs=exq, start=True, stop=True)
                nc.vector.tensor_copy(out=gst[0:8, s * CT + q, :], in_=pg)
            gm = gst[0:8, s * CT : (s + 1) * CT, 0:1]
            gx2 = gst[0:8, s * CT : (s + 1) * CT, 1:2]
            gv = stp.tile([8, CT, 1], f32, tag="gv")
            nc.vector.tensor_tensor(gv, gm, gm, ALU.mult)
            nc.vector.tensor_tensor(gv, gx2, gv, ALU.subtract)
            nc.vector.tensor_scalar(gv, gv, EPS, None, op0=ALU.add)
            # rstd = rsqrt(v) on DVE: quake seed + 3 Newton steps (keeps the
            # stats chain off ACT's table-reload path; ~1e-7 rel)
            i32 = dt.int32
            yb = stp.tile([8, CT, 1], f32, tag="yb")
            nc.vector.tensor_scalar(
                yb.bitcast(i32), gv.bitcast(i32), 1, None,
                op0=ALU.arith_shift_right,
            )
            nc.vector.tensor_tensor(
                yb.bitcast(i32), magic, yb.bitcast(i32), ALU.subtract
            )
            hh = stp.tile([8, CT, 1], f32, tag="hh")
            nc.vector.tensor_scalar(hh, gv, 0.5, None, op0=ALU.mult)
            ttn = stp.tile([8, CT, 1], f32, tag="ttn")
            for _ in range(3):
                nc.vector.tensor_tensor(ttn, yb, yb, ALU.mult)
                nc.vector.tensor_tensor(ttn, hh, ttn, ALU.mult)
                nc.vector.tensor_scalar(
                    ttn, ttn, -1.0, 1.5, op0=ALU.mult, op1=ALU.add
                )
                nc.vector.tensor_tensor(yb, yb, ttn, ALU.mult)
            gv = yb
            # replicate each group row to its 16 channel partitions
            rstdb = stp.tile([P, CT], f32, tag="rstdb")
            nc.scalar.dma_start(rstdb.opt(keep_dims={0}), pbc(gv[0:8, :, 0], 16))
            gmt = stp.tile([8, CT, 1], f32, tag="gmt")
            nc.vector.tensor_copy(out=gmt, in_=gm)
            gmb = stp.tile([P, CT], f32, tag="gmb")
            nc.scalar.dma_start(gmb.opt(keep_dims={0}), pbc(gmt[0:8, :, 0], 16))
            cs = scv[:, s * CT : (s + 1) * CT]
            nc.vector.tensor_tensor(cs, gw_sb, rstdb, ALU.mult)
            tmpb = stp.tile([P, CT], f32, tag="tmpb")
            nc.vector.tensor_tensor(tmpb, gmb, cs, ALU.mult)
            nc.vector.tensor_tensor(
                tcv[:, s * CT : (s + 1) * CT], gb_sb, tmpb, ALU.subtract
            )

        emit_stats(0)

        ptmp = ctx.enter_context(tc.tile_pool(name="ptmp", bufs=2))
        bigs = ctx.enter_context(tc.tile_pool(name="bigs", bufs=1))
        rbp = ctx.enter_context(tc.tile_pool(name="rbp", bufs=2))

        # ---- per-sample attention ----
        nts = {}

        def emit_normalize(s):
            nt = bigs.tile([P, CT, HW], mdt, tag="nt", bufs=2, name=f"nt{s}")
            nts[s] = nt
            for q in range(CT):
                nc.vector.tensor_scalar(
                    nt[:, q],
                    xts[s][:, q],
                    scv[:, s * CT + q : s * CT + q + 1],
                    tcv[:, s * CT + q : s * CT + q + 1],
                    op0=ALU.mult,
                    op1=ALU.add,
                )

        emit_normalize(0)

        for _s in (1, 2):
            xts[_s] = xtp.tile([P, CT, HW], f32, tag="xt", name=f"xt{_s}")
            nc.sync.dma_start(xts[_s], x_d[_s].rearrange("(q p) f -> p q f", p=P))
        pt_sb = singles.tile([P, CT, C], mdt)
        nc.sync.dma_start(pt_sb, pt_d.rearrange("(k p) o -> p k o", p=P))
        qks, vts, ess = {}, {}, {}

        def emit_qkv(s):
            # q' and k' [c, hw] (scale folded into weights, bias via ACT)
            nt = nts[s]
            qksb = bigs.tile([P, 8, HW], mdt_att, tag="qk", name=f"qk{s}")
            qks[s] = qksb
            for ot in range(8):
                for n in range(NH):
                    ps = pmm.tile([P, NF], f32, tag="mm")
                    for k in range(CT):
                        nc.tensor.matmul(
                            ps,
                            lhsT=wt_sb[:, k, ot * P : (ot + 1) * P],
                            rhs=nt[:, k, n * NF : (n + 1) * NF],
                            start=(k == 0),
                            stop=(k == CT - 1),
                        )
                    nc.scalar.activation(
                        out=qksb[:, ot, n * NF : (n + 1) * NF],
                        in_=ps,
                        func=AF.Identity,
                        bias=qkb_sb[:, ot : ot + 1],
                    )

        def emit_vt(s):
            # v computed transposed: [ij, c_v] (v bias folded into proj bias)
            nt = nts[s]
            vtsb = bigs.tile([P, IJT, C], mdt_att, tag="vt", name=f"vt{s}")
            vts[s] = vtsb
            for t in range(IJT):
                ps = pmm.tile([P, NF], f32, tag="mm")
                for k in range(CT):
                    nc.tensor.matmul(
                        ps,
                        lhsT=nt[:, k, t * P : (t + 1) * P],
                        rhs=wt_sb[:, k, 2 * C : 3 * C],
                        start=(k == 0),
                        stop=(k == CT - 1),
                    )
                nc.vector.tensor_copy(out=vtsb[:, t], in_=ps)

        def emit_scores_exp(s):
            # scores transposed S^T[ij, hw]; E = exp(S^T)
            qksb = qks[s]
            esb = bigs.tile([P, IJT, HW], mdt_att, tag="E", name=f"E{s}")
            ess[s] = esb
            for t in range(IJT):
                for n in range(NH):
                    ps = pmm.tile([P, NF], f32, tag="mm")
                    for k in range(CT):
                        nc.tensor.matmul(
                            ps,
                            lhsT=qksb[:, 4 + k, t * P : (t + 1) * P],
                            rhs=qksb[:, k, n * NF : (n + 1) * NF],
                            start=(k == 0),
                            stop=(k == CT - 1),
                        )
                    nc.scalar.activation(
                        out=esb[:, t, n * NF : (n + 1) * NF], in_=ps, func=AF.Exp
                    )

        def emit_softmax(s):
            # per-(i,hw) denominators D via select-matrix matmuls (sum the
            # 32 j-partitions, accumulating all 8 ij-tiles into one psum),
            # R = 1/D (custom DVE approx op, off ACT's table path), then
            # A^T = E * broadcast(R): replicate each i-row of R to its 32
            # j-partitions with a DMA and multiply on the idle GPSIMD engine
            esb = ess[s]
            dd = small.tile([G, HW], f32, tag="dd")
            rr = small.tile([G, HW], f32, tag="rr")
            rsc = small.tile([G, HW], f32, tag="rsc")
            for n in range(NH):
                pd = pdp.tile([G, NF], f32, tag="pd")
                for t in range(IJT):
                    nc.tensor.matmul(
                        pd,
                        lhsT=ss_sb[:, t, :],
                        rhs=esb[:, t, n * NF : (n + 1) * NF],
                        start=(t == 0),
                        stop=(t == IJT - 1),
                    )
                nc.vector.tensor_copy(out=dd[:, n * NF : (n + 1) * NF], in_=pd)
                nc.vector.reciprocal_approx_accurate(
                    out=rr[:, n * NF : (n + 1) * NF],
                    in_=dd[:, n * NF : (n + 1) * NF],
                    scratch=rsc[:, n * NF : (n + 1) * NF],
                )
            for t in range(IJT):
                for n in range(NH):
                    rbt = rbp.tile([P, NF], f32, tag="rb")
                    nc.sync.dma_start(
                        rbt, pbc(rr[4 * t : 4 * t + 4, n * NF : (n + 1) * NF], 32)
                    )
                    nc.gpsimd.tensor_tensor(
                        esb[:, t, n * NF : (n + 1) * NF],
                        esb[:, t, n * NF : (n + 1) * NF],
                        rbt,
                        ALU.mult,
                    )

        def emit_h(s):
            # h[c, hw] = sum_ij v^T[ij,c] * A^T[ij,hw]  (h overwrites nt)
            nt, vtsb, esb = nts[s], vts[s], ess[s]
            for ct in range(CT):
                for n in range(NH):
                    ps = pmm.tile([P, NF], f32, tag="mm")
                    for t in range(IJT):
                        nc.tensor.matmul(
                            ps,
                            lhsT=vtsb[:, t, ct * P : (ct + 1) * P],
                            rhs=esb[:, t, n * NF : (n + 1) * NF],
                            start=(t == 0),
                            stop=(t == IJT - 1),
                        )
                    nc.scalar.activation(
                        out=nt[:, ct, n * NF : (n + 1) * NF], in_=ps, func=AF.Identity
                    )

        def emit_proj(s):
            # proj + bias + residual (accumulated into xt), then store
            nt, xt = nts[s], xts[s]
            ov = out_d[s].rearrange("(q p) f -> p q f", p=P)
            for ot in range(CT):
                for n in range(NH):
                    ps = pmm.tile([P, NF], f32, tag="mm")
                    for k in range(CT):
                        nc.tensor.matmul(
                            ps,
                            lhsT=pt_sb[:, k, ot * P : (ot + 1) * P],
                            rhs=nt[:, k, n * NF : (n + 1) * NF],
                            start=(k == 0),
                            stop=(k == CT - 1),
                        )
                    tmp = ptmp.tile([P, NF], f32, tag="pt")
                    nc.scalar.activation(
                        out=tmp, in_=ps, func=AF.Identity, bias=pb_sb[:, ot : ot + 1]
                    )
                    nc.vector.tensor_tensor(
                        xt[:, ot, n * NF : (n + 1) * NF],
                        xt[:, ot, n * NF : (n + 1) * NF],
                        tmp,
                        ALU.add,
                    )
                nc.gpsimd.dma_start(ov[:, ot], xt[:, ot])

        # software pipeline: the next sample's qkv/vT matmuls are emitted
        # inside this sample's softmax window so the PE never waits for the
        # softmax chain (D -> 1/D -> broadcast -> A-mul) to complete
        emit_qkv(0)
        emit_vt(0)
        projected = set()
        for s in range(BS):
            emit_scores_exp(s)
            if s + 1 < BS:
                if s + 1 not in xts:
                    xts[s + 1] = xtp.tile(
                        [P, CT, HW], f32, tag="xt", name=f"xt{s + 1}"
                    )
                    nc.sync.dma_start(
                        xts[s + 1],
                        x_d[s + 1].rearrange("(q p) f -> p q f", p=P),
                    )
                emit_stats(s + 1)
                emit_normalize(s + 1)
            emit_softmax(s)
            if s + 1 < BS:
                emit_qkv(s + 1)
            elif s >= 1 and s - 1 not in projected:
                # last sample has no next-qkv filler: cover its softmax
                # chain with the previous sample's deferred proj
                projected.add(s - 1)
                emit_proj(s - 1)
            emit_h(s)
            if s + 1 < BS:
                emit_vt(s + 1)
            if s != BS - 2:
                projected.add(s)
                emit_proj(s)

    nc.compile()
    return nc


def _prep_inputs(x, gn_w, gn_b, qkv_w, qkv_b, proj_w, proj_b):
    x = np.asarray(x, dtype=np.float32)
    gn_w = np.asarray(gn_w, dtype=np.float32)
    gn_b = np.asarray(gn_b, dtype=np.float32)
    qkv_w = np.asarray(qkv_w, dtype=np.float32)
    qkv_b = np.asarray(qkv_b, dtype=np.float32)
    proj_w = np.asarray(proj_w, dtype=np.float32)
    proj_b = np.asarray(proj_b, dtype=np.float32)

    s4 = np.float32(float(C) ** -0.25)
    w_s = qkv_w.copy()
    w_s[: 2 * C] *= s4
    wt = np.ascontiguousarray(w_s.T)                     # [C, 3C]
    pt = np.ascontiguousarray(proj_w.T)                  # [C, C]
    qkb = np.ascontiguousarray((qkv_b[: 2 * C] * s4).reshape(8, P).T)  # [P, 8]
    vb = qkv_b[2 * C :]
    pb = np.ascontiguousarray(
        (proj_b + np.float32(H) * (proj_w @ vb)).reshape(CT, P).T
    )                                                    # [P, CT]
    gw = np.ascontiguousarray(gn_w.reshape(CT, P).T)   # [P, CT]
    gb = np.ascontiguousarray(gn_b.reshape(CT, P).T)
    selg = np.zeros((P, 8), dtype=np.float32)
    selg[np.arange(P), np.arange(P) // 16] = 1.0 / 16.0
    ss = np.zeros((P, IJT, G), dtype=np.float32)
    for t in range(IJT):
        for p in range(P):
            ss[p, t, 4 * t + p // 32] = 1.0
    ss = np.ascontiguousarray(ss.reshape(P, IJT * G))
    shared = {
        "wt": wt, "pt": pt, "qkb": qkb, "pb": pb,
        "gw": gw, "gb": gb, "ssum": ss, "selg": selg,
    }
    in_maps = []
    for c in range(NCORES):
        m = dict(shared)
        m["x"] = np.ascontiguousarray(x[c * BS : (c + 1) * BS].reshape(BS, C, HW))
        in_maps.append(m)
    return in_maps


def run(inputs: dict, trace: bool = False, n_cores: int = NCORES):
    """Build (cached), run on hardware, return (results, BassKernelResults)."""
    from concourse.bass_utils import run_bass_kernel_spmd

    key = MM_MODE
    if key not in _cache:
        _cache[key] = _build(MM_MODE)
    nc = _cache[key]
    in_maps = _prep_inputs(**inputs)[:n_cores]
    res = run_bass_kernel_spmd(nc, in_maps, list(range(n_cores)), trace=trace)
    return res


def kernel(x, gn_w, gn_b, qkv_w, qkv_b, proj_w, proj_b) -> np.ndarray:
    res = run(dict(x=x, gn_w=gn_w, gn_b=gn_b, qkv_w=qkv_w, qkv_b=qkv_b,
                   proj_w=proj_w, proj_b=proj_b))
    out = np.concatenate(
        [res.results[c]["out"].reshape(BS, C, H, W) for c in range(NCORES)], axis=0
    )
    return out
